# revision 1
# baseline (speedup 1.0000x reference)
"""CrossNeighborAttention Trainium2 kernel (8 NeuronCores, SPMD data-parallel).

Sharding: 16 (b,t) frames over 8 cores -> 2 frames/core. Neighbor-frame K/V
("halo") is handled host-side: each core receives the img of 4 frame-slots
ordered [lo, own1, own0, hi], so frame f's keys are the contiguous slot pair
[f*1152 : f*1152+1152] in (prev|next) order and the device program is
identical on every core (pure SPMD, no collectives).

Per core:
  - xT is host-pretransposed to [C, tokens]; Q/K projections produce
    [c_out, token] tiles directly (f32r matmuls, full PE rate at N>=256).
  - RoPE: k*cos + P@(sin_swapped*k); the pair rotation P is a 128x128 matmul,
    sin_swapped is host-precomputed; the add runs in place on the K/Q tile.
  - V is projected in [token, c_out] layout with a ones-column appended per
    head (65-wide groups) so the AV matmul emits the softmax denominator as
    output row 64 for free.
  - scores S^T[key, q] accumulate in PSUM; exp on ACT with the 1/8 softmax
    scale fused (6-bank groups); A^T stored bf16; AV in bf16.
  - 1/denom = exp(-ln(denom)) on ACT, broadcast over the 64 head dims via a
    K=1 ones-matmul, applied by one DVE multiply per head.
  - output projection accumulates over head-pair chunks; the bias is folded
    in as a K=1 ones x bo matmul in the same PSUM accumulation group.
"""

import sys

for p in ("/opt/trn_rl_repo", "/opt/trn_rl_repo/concourse"):
    if p not in sys.path:
        sys.path.insert(0, p)

import numpy as np

import concourse.bass as bass
import concourse.mybir as mybir
import concourse.tile as tile
from concourse import bacc
from concourse.bass_utils import run_bass_kernel_spmd

F32 = mybir.dt.float32
F32R = mybir.dt.float32r
BF16 = mybir.dt.bfloat16
FP16 = mybir.dt.float16
EXP = mybir.ActivationFunctionType.Exp
LN = mybir.ActivationFunctionType.Ln

B, T, N, C, H = 2, 8, 576, 1024, 16
HD = C // H           # 64
NCORES = 8
SLN = N               # 576 tokens per frame-slot
NKV = 2 * N           # 1152 kv tokens per frame
CH = 288              # attention query chunk (PSUM bank holds 512 fp32)
KCH = 384             # projection token chunk (>=256 keeps f32r full rate)
KK = NKV // 128       # 9 key tiles per frame
NCI = C // 128        # 8 contraction chunks
NHP = H // 2          # 8 head-pairs (c_out chunks of 128)


def _patch_act_tables():
    """Make natural_log_exp_and_others the only set containing Exp and Ln so
    the table-load pass doesn't thrash between exp-only and ln-only sets
    (each reload costs ~2.7us on ACT). Mutates the functools.cache'd dict
    in place; set order/indices are preserved for walrus remapping."""
    import concourse.hw_specs as hw_specs

    t = hw_specs.get_activation_tables("gen3")
    for name, fns in t.items():
        if name != "natural_log_exp_and_others":
            fns.discard(EXP)
            fns.discard(LN)


def _build_nc(iters: int, skip=()):
    _patch_act_tables()
    nc = bacc.Bacc("TRN2", target_bir_lowering=False)

    xT_d = nc.dram_tensor("xT", [C, 4 * SLN], F32, kind="ExternalInput")
    cosT_d = nc.dram_tensor("cosT", [128, 4 * SLN], F32, kind="ExternalInput")
    sinswT_d = nc.dram_tensor("sinswT", [128, 4 * SLN], F32, kind="ExternalInput")
    wq_d = nc.dram_tensor("wqT", [C, C], F32, kind="ExternalInput")
    wk_d = nc.dram_tensor("wkT", [C, C], F32, kind="ExternalInput")
    wv_d = nc.dram_tensor("wvT", [C, C], F32, kind="ExternalInput")
    wo_d = nc.dram_tensor("woT", [C, C], F32, kind="ExternalInput")
    pt_d = nc.dram_tensor("pT", [128, 128], F32, kind="ExternalInput")
    bo_d = nc.dram_tensor("bo", [1, C], F32, kind="ExternalInput")
    out_d = nc.dram_tensor("out", [2 * SLN, C], F32, kind="ExternalOutput")

    def wquarter_ap(w_d, q):
        # rows [q*256, (q+1)*256) of the [C, C] weight as [128, 2, C]
        return bass.AP(
            tensor=w_d.ap().tensor,
            offset=q * 2 * 128 * C,
            ap=[[C, 128], [128 * C, 2], [1, C]],
        )

    with tile.TileContext(nc) as tc:
        import contextlib

        ctx = contextlib.ExitStack()
        with ctx:
            const_p = ctx.enter_context(tc.tile_pool(name="const", bufs=1))
            w_p = ctx.enter_context(tc.tile_pool(name="wp", bufs=5))
            x_p = ctx.enter_context(tc.tile_pool(name="xp", bufs=1))
            freq_p = ctx.enter_context(tc.tile_pool(name="freq", bufs=1))
            kt_p = ctx.enter_context(tc.tile_pool(name="ktp", bufs=1))
            v_p = ctx.enter_context(tc.tile_pool(name="vp", bufs=1))
            qt_p = ctx.enter_context(tc.tile_pool(name="qtp", bufs=1))
            ot_p = ctx.enter_context(tc.tile_pool(name="otp", bufs=1))
            at_p = ctx.enter_context(tc.tile_pool(name="atp", bufs=1))
            st_p = ctx.enter_context(tc.tile_pool(name="stage", bufs=1))
            outst_p = ctx.enter_context(tc.tile_pool(name="outst", bufs=2))

            # Constants
            pT = const_p.tile([128, 128], F32R, name="pT")
            nc.gpsimd.dma_start(out=pT, in_=pt_d[:, :])
            bo_sb = const_p.tile([1, C], F32, name="bo_sb")
            nc.sync.dma_start(out=bo_sb, in_=bo_d[:, :])
            ones64 = const_p.tile([1, 64], F32, name="ones64")
            nc.vector.memset(ones64, 1.0)
            ones128 = const_p.tile([1, 128], F32, name="ones128")
            nc.vector.memset(ones128, 1.0)

            def load_w(w_d, nm):
                w = [w_p.tile([128, 2, C], F32R, name=f"{nm}_{q}", tag="w")
                     for q in range(4)]
                for q in range(4):
                    nc.gpsimd.dma_start(out=w[q], in_=wquarter_ap(w_d, q))
                return w

            def proj_rope(psp, f, w, dest, chunks, xsrc, cosS, sinS, pfx):
                """dest[hp][:, :] = rope(W @ xsrc); chunks = token chunk sizes."""
                off = 0
                for ch, cw in enumerate(chunks):
                    xsl = slice(off, off + cw)
                    off += cw
                    for hp in range(NHP):
                        kp = psp.tile([128, cw], F32, name=f"p{pfx}{f}_{ch}_{hp}", tag="proj", padded_shape=[128, 512])
                        for ci in range(NCI):
                            nc.tensor.matmul(
                                kp, w[ci // 2][:, ci % 2, hp * 128:(hp + 1) * 128],
                                xsrc[:, ci, xsl], start=(ci == 0), stop=(ci == NCI - 1))
                        m = st_p.tile([128, cw], F32R, name=f"m{pfx}{f}_{ch}_{hp}", tag="m", padded_shape=[128, 512])
                        nc.vector.tensor_mul(m, kp, sinS[:, xsl])
                        r = psp.tile([128, cw], F32, name=f"r{pfx}{f}_{ch}_{hp}", tag="rot", padded_shape=[128, 512])
                        nc.tensor.matmul(r, pT, m, start=True, stop=True)
                        nc.vector.tensor_mul(dest[hp][:, xsl], kp, cosS[:, xsl])
                        nc.vector.tensor_add(dest[hp][:, xsl], dest[hp][:, xsl], r)

            def body():
                for f in range(2):
                    kv0 = f * NKV            # kv token base in slot space
                    q0 = 1152 - 576 * f      # own-frame token base in slot space

                    # ================= projections =================
                    with tc.tile_pool(name=f"psp{f}", bufs=2, space="PSUM") as psp:
                        wk = load_w(wk_d, f"wk{f}")
                        cosK = freq_p.tile([128, NKV], F32, name=f"cosK{f}", tag="cosS")
                        sinK = freq_p.tile([128, NKV], F32, name=f"sinK{f}", tag="sinS")
                        nc.sync.dma_start(out=cosK, in_=cosT_d[:, kv0:kv0 + NKV])
                        nc.sync.dma_start(out=sinK, in_=sinswT_d[:, kv0:kv0 + NKV])
                        xkv = x_p.tile([128, NCI, NKV], F32R, name=f"xkv{f}", tag="x")
                        for ci in range(NCI):
                            nc.gpsimd.dma_start(
                                out=xkv[:, ci, :],
                                in_=xT_d[ci * 128:(ci + 1) * 128, kv0:kv0 + NKV])

                        K_T = [kt_p.tile([128, NKV], F32R, name=f"KT{f}_{hp}", tag=f"KT{hp}")
                               for hp in range(NHP)]
                        if "kproj" not in skip:
                            proj_rope(psp, f, wk, K_T, [KCH, KCH, KCH], xkv, cosK, sinK, "k")
                        else:
                            kstub = st_p.tile([128, NKV], F32, name=f"kstub{f}", tag="kstub")
                            nc.vector.memset(kstub, 0.01)
                            for hp in range(NHP):
                                nc.vector.tensor_copy(K_T[hp], kstub)

                        # ---- V projection (slices xkv as stationary) ----
                        wv = load_w(wv_d, f"wv{f}")
                        V = [v_p.tile([128, 16 * 65], BF16, name=f"V{f}_{tt}", tag=f"V{tt}")
                             for tt in range(KK)]
                        for tt in range(KK):
                            v3 = V[tt].rearrange("p (g e) -> p g e", e=65)
                            nc.vector.memset(v3[:, :, 64:65], 1.0)
                            for co in range(2):
                                vp = psp.tile([128, 512], F32, name=f"vp{f}_{tt}_{co}", tag="vproj")
                                for ci in range(NCI):
                                    nc.tensor.matmul(
                                        vp, xkv[:, ci, tt * 128:(tt + 1) * 128],
                                        wv[ci // 2][:, ci % 2, co * 512:(co + 1) * 512],
                                        start=(ci == 0), stop=(ci == NCI - 1))
                                nc.vector.tensor_copy(
                                    v3[:, co * 8:(co + 1) * 8, 0:64],
                                    vp.rearrange("p (h d) -> p h d", d=64))

                        # ---- Q projection + rope ----
                        wq = load_w(wq_d, f"wq{f}")
                        cosQ = freq_p.tile([128, SLN], F32, name=f"cosQ{f}", tag="cosS")
                        sinQ = freq_p.tile([128, SLN], F32, name=f"sinQ{f}", tag="sinS")
                        nc.sync.dma_start(out=cosQ, in_=cosT_d[:, q0:q0 + SLN])
                        nc.sync.dma_start(out=sinQ, in_=sinswT_d[:, q0:q0 + SLN])
                        xq = x_p.tile([128, NCI, SLN], F32R, name=f"xq{f}", tag="x")
                        for ci in range(NCI):
                            nc.gpsimd.dma_start(
                                out=xq[:, ci, :],
                                in_=xT_d[ci * 128:(ci + 1) * 128, q0:q0 + SLN])
                        Q_T = [qt_p.tile([128, SLN], F32R, name=f"QT{f}_{hp}", tag=f"QT{hp}")
                               for hp in range(NHP)]
                        proj_rope(psp, f, wq, Q_T, [CH, CH], xq, cosQ, sinQ, "q")

                    # ================= attention =================
                    O_T = [ot_p.tile([128, SLN], F32R, name=f"OT{f}_{hp}", tag=f"OT{hp}")
                           for hp in range(NHP)]
                    if "att" in skip:
                        for hp in range(NHP):
                            nc.vector.tensor_copy(O_T[hp], K_T[hp][:, 0:SLN])
                    else:
                        with tc.tile_pool(name=f"psa{f}", bufs=1, space="PSUM") as psa:
                            S = psa.tile([128, 6, CH], F32, name=f"S{f}", tag="S",
                                         padded_shape=[128, 6, 512])
                            Op = psa.tile([128, 2, CH], F32, name=f"Opp{f}", tag="Op",
                                          padded_shape=[128, 2, 512])
                            for hp in range(NHP):
                                for hh in range(2):
                                    hs = slice(hh * 64, hh * 64 + 64)
                                    g = 2 * hp + hh  # global head
                                    aT = at_p.tile([128, KK, SLN], BF16,
                                                   name=f"aT{f}_{hp}_{hh}", tag="aT")
                                    aTf = aT.rearrange("p kk n -> p (kk n)")
                                    for l in range(18):   # kk major, cq minor
                                        kk, cq = divmod(l, 2)
                                        slot = l % 6
                                        nc.tensor.matmul(
                                            S[:, slot, :],
                                            K_T[hp][hs, kk * 128:(kk + 1) * 128],
                                            Q_T[hp][hs, cq * CH:(cq + 1) * CH],
                                            start=True, stop=True)
                                        if slot == 5:   # exp one 6-bank group
                                            nc.scalar.activation(
                                                aTf[:, (l - 5) * CH:(l + 1) * CH],
                                                S[:, 0:6, :], EXP, scale=0.125)
                                    for cq in range(2):
                                        for kk in range(KK):
                                            nc.tensor.matmul(
                                                Op[0:65, cq, :],
                                                V[kk][:, g * 65:(g + 1) * 65],
                                                aT[:, kk, cq * CH:(cq + 1) * CH],
                                                start=(kk == 0), stop=(kk == KK - 1))
                                    # 1/denom = exp(-ln(denom)), batched over cq
                                    ln_row = st_p.tile([1, 2, CH], F32,
                                                       name=f"ln{f}_{g}", tag="ln")
                                    recipB = st_p.tile([64, 2, CH], F32,
                                                       name=f"rb{f}_{g}", tag="rb")
                                    nc.scalar.activation(ln_row, Op[64:65, :, :], LN)
                                    for cq in range(2):
                                        nc.tensor.matmul(S[0:64, 2 + cq, :], ones64,
                                                         ln_row[:, cq, :],
                                                         start=True, stop=True)
                                    nc.scalar.activation(
                                        recipB, S[0:64, 2:4, :], EXP, scale=-1.0)
                                    nc.vector.tensor_mul(
                                        O_T[hp][hs, :].rearrange("p (c n) -> p c n", n=CH),
                                        Op[0:64, :, :], recipB)

                    # ================= output projection =================
                    with tc.tile_pool(name=f"pso{f}", bufs=2, space="PSUM") as pso:
                        wo = load_w(wo_d, f"wo{f}")
                        for t0, tl in [(0, 128), (128, 128), (256, 128), (384, 128), (512, 64)]:
                            for co in range(2):
                                op = pso.tile([128, 512], F32,
                                              name=f"op{f}_{t0}_{co}", tag="oproj")
                                for hp in range(NHP):
                                    nc.tensor.matmul(
                                        op[0:tl, :], O_T[hp][:, t0:t0 + tl],
                                        wo[hp // 2][:, hp % 2, co * 512:(co + 1) * 512],
                                        start=(hp == 0), stop=False)
                                # fold the bias in as ones^T @ bo
                                nc.tensor.matmul(
                                    op[0:tl, :], ones128[:, 0:tl],
                                    bo_sb[:, co * 512:(co + 1) * 512],
                                    start=False, stop=True)
                                outst = outst_p.tile([128, 512], F32,
                                                     name=f"os{f}_{t0}_{co}", tag="os")
                                nc.vector.tensor_copy(outst[0:tl, :], op[0:tl, :])
                                nc.sync.dma_start(
                                    out=out_d[f * SLN + t0: f * SLN + t0 + tl,
                                              co * 512:(co + 1) * 512],
                                    in_=outst[0:tl, :])

            if iters > 1:
                with tc.For_i(0, iters, 1):
                    body()
            else:
                body()

    nc.compile()
    return nc


_nc_cache = {}


def _get_nc(iters=1, skip=()):
    key = (iters, tuple(skip))
    if key not in _nc_cache:
        _nc_cache[key] = _build_nc(iters, skip)
    return _nc_cache[key]


def _host_prep(img, freqs_cos, freqs_sin, Wq, Wk, Wv, Wo, bo):
    img = np.asarray(img, dtype=np.float32)
    freqs_cos = np.asarray(freqs_cos, dtype=np.float32)
    freqs_sin = np.asarray(freqs_sin, dtype=np.float32)

    # pair-swapped sin: sinsw[2i] = sin[2i+1], sinsw[2i+1] = sin[2i]
    sw = np.arange(HD).reshape(-1, 2)[:, ::-1].reshape(-1)
    sinsw = freqs_sin[:, sw]

    cos_f = freqs_cos.reshape(T, N, HD)
    sin_f = sinsw.reshape(T, N, HD)

    # rot(x)[2i] = -x[2i+1], rot(x)[2i+1] = x[2i]; as rot = P @ x (per 64-dim
    # head half, tiled to 128); the matmul takes P^T as the stationary side.
    P = np.zeros((128, 128), np.float32)
    for i in range(64):
        P[2 * i, 2 * i + 1] = -1.0
        P[2 * i + 1, 2 * i] = 1.0
    pT = np.ascontiguousarray(P.T)

    wqT = np.ascontiguousarray(np.asarray(Wq, np.float32).T)
    wkT = np.ascontiguousarray(np.asarray(Wk, np.float32).T)
    wvT = np.ascontiguousarray(np.asarray(Wv, np.float32).T)
    woT = np.ascontiguousarray(np.asarray(Wo, np.float32).T)
    bo2 = np.asarray(bo, dtype=np.float32).reshape(1, C)

    in_maps = []
    for core in range(NCORES):
        b, fp = divmod(core, 4)
        own0, own1 = 2 * fp, 2 * fp + 1
        lo = own0 - 1 if fp > 0 else 1
        hi = own1 + 1 if fp < 3 else 6
        slots = [lo, own1, own0, hi]
        xT = np.ascontiguousarray(img[b, slots].reshape(4 * N, C).T)
        cosT = np.ascontiguousarray(cos_f[slots].reshape(4 * N, HD).T)
        sinT = np.ascontiguousarray(sin_f[slots].reshape(4 * N, HD).T)
        in_maps.append({
            "xT": xT,
            "cosT": np.concatenate([cosT, cosT], axis=0),
            "sinswT": np.concatenate([sinT, sinT], axis=0),
            "wqT": wqT, "wkT": wkT, "wvT": wvT, "woT": woT,
            "pT": pT, "bo": bo2,
        })
    return in_maps


def kernel(img, freqs_cos, freqs_sin, Wq, Wk, Wv, Wo, bo, _iters=1, _skip=()):
    in_maps = _host_prep(img, freqs_cos, freqs_sin, Wq, Wk, Wv, Wo, bo)
    nc = _get_nc(_iters, _skip)
    res = run_bass_kernel_spmd(nc, in_maps, core_ids=list(range(NCORES)))
    out = np.zeros((B, T, N, C), np.float32)
    for core in range(NCORES):
        b, fp = divmod(core, 4)
        r = res.results[core]["out"].reshape(2, N, C)
        out[b, 2 * fp] = r[0]
        out[b, 2 * fp + 1] = r[1]
    return out



# revision 9
# speedup vs baseline: 1.4537x; 1.4537x over previous
"""CrossNeighborAttention Trainium2 kernel (8 NeuronCores, SPMD data-parallel).

Sharding: 16 (b,t) frames over 8 cores -> 2 frames/core. Neighbor-frame K/V
("halo") is handled host-side: each core receives the img of 4 frame-slots
ordered [lo, own1, own0, hi], so frame f's keys are the contiguous slot pair
[f*1152 : f*1152+1152] in (prev|next) order and the device program is
identical on every core (pure SPMD, no collectives).

v2 layout (vs v1): all inputs bf16 (halves DMA + SBUF); x / cos / sin loaded
ONCE per iteration as a single 2304-token panel (q columns are slices of the
kv panel, no separate q loads); weights loaded once (not per frame). The
attention inner loop is software-pipelined: exp runs on 3-PSUM-bank groups
and AV matmuls are interleaved between S-matmul groups so ACT exp overlaps
PE work instead of serializing with it. The softmax-denominator broadcast
(ones64 @ ln_row) uses an f32r moving operand for full PE rate.

Per core:
  - xT is host-pretransposed to [C, tokens]; Q/K projections produce
    [c_out, token] tiles directly.
  - RoPE: k*cos + P@(sin_swapped*k); the pair rotation P is a 128x128 matmul,
    sin_swapped is host-precomputed; the add runs in place on the K/Q tile.
  - V is projected in [token, c_out] layout with a ones-column appended per
    head (65-wide groups) so the AV matmul emits the softmax denominator as
    output row 64 for free.
  - scores S^T[key, q] accumulate in PSUM; exp on ACT with the 1/8 softmax
    scale fused (3-bank groups); A^T stored bf16; AV in bf16.
  - 1/denom = exp(-ln(denom)) on ACT, broadcast over the 64 head dims via a
    K=1 ones-matmul, applied by one DVE multiply per head.
  - output projection accumulates over head-pair chunks; the bias is folded
    in as a K=1 ones x bo matmul in the same PSUM accumulation group.
"""

import sys

for p in ("/opt/trn_rl_repo", "/opt/trn_rl_repo/concourse"):
    if p not in sys.path:
        sys.path.insert(0, p)

import numpy as np

import concourse.bass as bass
import concourse.mybir as mybir
import concourse.tile as tile
from concourse import bacc
from concourse.bass_utils import run_bass_kernel_spmd

F32 = mybir.dt.float32
F32R = mybir.dt.float32r
BF16 = mybir.dt.bfloat16
EXP = mybir.ActivationFunctionType.Exp
LN = mybir.ActivationFunctionType.Ln

B, T, N, C, H = 2, 8, 576, 1024, 16
HD = C // H           # 64
NCORES = 8
SLN = N               # 576 tokens per frame-slot
NKV = 2 * N           # 1152 kv tokens per frame
CH = 288              # attention query chunk (PSUM bank holds 512 fp32)
KK = NKV // 128       # 9 key tiles per frame
NCI = C // 128        # 8 contraction chunks
NHP = H // 2          # 8 head-pairs (c_out chunks of 128)


def _patch_act_tables():
    """Make natural_log_exp_and_others the only set containing Exp and Ln so
    the table-load pass doesn't thrash between exp-only and ln-only sets
    (each reload costs ~2.7us on ACT). Mutates the functools.cache'd dict
    in place; set order/indices are preserved for walrus remapping."""
    import concourse.hw_specs as hw_specs

    t = hw_specs.get_activation_tables("gen3")
    for name, fns in t.items():
        if name != "natural_log_exp_and_others":
            fns.discard(EXP)
            fns.discard(LN)


def _build_nc(iters: int, skip=()):
    _patch_act_tables()
    nc = bacc.Bacc("TRN2", target_bir_lowering=False)

    xT_d = nc.dram_tensor("xT", [C, 4 * SLN], BF16, kind="ExternalInput")
    cosT_d = nc.dram_tensor("cosT", [128, 4 * SLN], BF16, kind="ExternalInput")
    sinswT_d = nc.dram_tensor("sinswT", [128, 4 * SLN], BF16, kind="ExternalInput")
    wq_d = nc.dram_tensor("wqT", [C, C], BF16, kind="ExternalInput")
    wk_d = nc.dram_tensor("wkT", [C, C], BF16, kind="ExternalInput")
    wv_d = nc.dram_tensor("wvT", [C, C], BF16, kind="ExternalInput")
    wo_d = nc.dram_tensor("woT", [C, C], BF16, kind="ExternalInput")
    pt_d = nc.dram_tensor("pT", [128, 128], F32, kind="ExternalInput")
    bo_d = nc.dram_tensor("bo", [1, C], F32, kind="ExternalInput")
    out_d = nc.dram_tensor("out", [2 * SLN, C], F32, kind="ExternalOutput")

    def wquarter_ap(w_d, q):
        # rows [q*256, (q+1)*256) of the [C, C] weight as [128, 2, C]
        return bass.AP(
            tensor=w_d.ap().tensor,
            offset=q * 2 * 128 * C,
            ap=[[C, 128], [128 * C, 2], [1, C]],
        )

    with tile.TileContext(nc) as tc:
        import contextlib

        ctx = contextlib.ExitStack()
        with ctx:
            const_p = ctx.enter_context(tc.tile_pool(name="const", bufs=1))
            w_p = ctx.enter_context(tc.tile_pool(name="wp", bufs=1))
            x_p = ctx.enter_context(tc.tile_pool(name="xp", bufs=1))
            freq_p = ctx.enter_context(tc.tile_pool(name="freq", bufs=1))
            kt_p = ctx.enter_context(tc.tile_pool(name="ktp", bufs=1))
            v_p = ctx.enter_context(tc.tile_pool(name="vp", bufs=1))
            qt_p = ctx.enter_context(tc.tile_pool(name="qtp", bufs=1))
            ot_p = ctx.enter_context(tc.tile_pool(name="otp", bufs=1))
            at_p = ctx.enter_context(tc.tile_pool(name="atp", bufs=2))
            st_p = ctx.enter_context(tc.tile_pool(name="stage", bufs=1))
            outst_p = ctx.enter_context(tc.tile_pool(name="outst", bufs=2))

            # Constants
            pT = const_p.tile([128, 128], F32R, name="pT")
            nc.gpsimd.dma_start(out=pT, in_=pt_d[:, :])
            bo_sb = const_p.tile([1, C], BF16, name="bo_sb")
            nc.gpsimd.dma_start(out=bo_sb, in_=bo_d[:, :])
            ones128 = const_p.tile([1, 128], BF16, name="ones128")
            nc.vector.memset(ones128, 1.0)

            def load_w(w_d, nm):
                w = [w_p.tile([128, 2, C], BF16, name=f"{nm}_{q}", tag=f"w{nm}{q}")
                     for q in range(4)]
                for q in range(4):
                    nc.gpsimd.dma_start(out=w[q], in_=wquarter_ap(w_d, q))
                return w

            def proj_rope(psp, f, w, dest, chunks, xsrc, xoff, cosS, sinS, pfx):
                """dest[hp][:, :] = rope(W @ xsrc cols [xoff, xoff+sum(chunks)))."""
                off = 0
                for ch, cw in enumerate(chunks):
                    xsl = slice(xoff + off, xoff + off + cw)
                    dsl = slice(off, off + cw)
                    off += cw
                    for hp in range(NHP):
                        kp = psp.tile([128, cw], F32, name=f"p{pfx}{f}_{ch}_{hp}", tag="proj", padded_shape=[128, 512])
                        for ci in range(NCI):
                            nc.tensor.matmul(
                                kp, w[ci // 2][:, ci % 2, hp * 128:(hp + 1) * 128],
                                xsrc[:, ci, xsl], start=(ci == 0), stop=(ci == NCI - 1))
                        m = st_p.tile([128, cw], F32R, name=f"m{pfx}{f}_{ch}_{hp}", tag="m", padded_shape=[128, 512])
                        nc.vector.tensor_mul(m, kp, sinS[:, xsl])
                        r = psp.tile([128, cw], F32, name=f"r{pfx}{f}_{ch}_{hp}", tag="rot", padded_shape=[128, 512])
                        nc.tensor.matmul(r, pT, m, start=True, stop=True)
                        nc.vector.tensor_mul(dest[hp][:, dsl], kp, cosS[:, xsl])
                        nc.vector.tensor_add(dest[hp][:, dsl], dest[hp][:, dsl], r)

            def body():
                # ---- all loop-variant inputs, loaded once up front ----
                xall = x_p.tile([128, NCI, 2 * NKV], BF16, name="xall", tag="x")
                for ci in range(NCI):
                    nc.gpsimd.dma_start(
                        out=xall[:, ci, :],
                        in_=xT_d[ci * 128:(ci + 1) * 128, :])
                cosA = freq_p.tile([128, 2 * NKV], BF16, name="cosA", tag="cos")
                sinA = freq_p.tile([128, 2 * NKV], BF16, name="sinA", tag="sin")
                nc.gpsimd.dma_start(out=cosA, in_=cosT_d[:, :])
                nc.gpsimd.dma_start(out=sinA, in_=sinswT_d[:, :])
                wk = load_w(wk_d, "wk")
                wv = load_w(wv_d, "wv")
                wq = load_w(wq_d, "wq")
                wo = load_w(wo_d, "wo")

                for f in range(2):
                    kv0 = f * NKV            # kv token base in slot space
                    q0 = 1152 - 576 * f      # own-frame token base in slot space

                    # ================= projections =================
                    with tc.tile_pool(name=f"psp{f}", bufs=2, space="PSUM") as psp:
                        K_T = [kt_p.tile([128, NKV], BF16, name=f"KT{f}_{hp}", tag=f"KT{hp}")
                               for hp in range(NHP)]
                        if "kproj" not in skip:
                            proj_rope(psp, f, wk, K_T, [384, 384, 384], xall,
                                      kv0, cosA, sinA, "k")
                        else:
                            kstub = st_p.tile([128, NKV], BF16, name=f"kstub{f}", tag="kstub")
                            nc.vector.memset(kstub, 0.01)
                            for hp in range(NHP):
                                nc.vector.tensor_copy(K_T[hp], kstub)

                        # ---- V projection (slices xall as stationary) ----
                        V = [v_p.tile([128, 16 * 65], BF16, name=f"V{f}_{tt}", tag=f"V{tt}")
                             for tt in range(KK)]
                        for tt in range(KK):
                            v3 = V[tt].rearrange("p (g e) -> p g e", e=65)
                            nc.vector.memset(v3[:, :, 64:65], 1.0)
                            for co in range(2):
                                vp = psp.tile([128, 512], F32, name=f"vp{f}_{tt}_{co}", tag="proj")
                                for ci in range(NCI):
                                    nc.tensor.matmul(
                                        vp, xall[:, ci, kv0 + tt * 128:kv0 + (tt + 1) * 128],
                                        wv[ci // 2][:, ci % 2, co * 512:(co + 1) * 512],
                                        start=(ci == 0), stop=(ci == NCI - 1))
                                nc.vector.tensor_copy(
                                    v3[:, co * 8:(co + 1) * 8, 0:64],
                                    vp.rearrange("p (h d) -> p h d", d=64))

                        # ---- Q projection + rope ----
                        Q_T = [qt_p.tile([128, SLN], BF16, name=f"QT{f}_{hp}", tag=f"QT{hp}")
                               for hp in range(NHP)]
                        proj_rope(psp, f, wq, Q_T, [CH, CH], xall, q0, cosA, sinA, "q")

                    # ================= attention =================
                    O_T = [ot_p.tile([128, SLN], BF16, name=f"OT{f}_{hp}", tag=f"OT{hp}")
                           for hp in range(NHP)]
                    if "att" in skip:
                        for hp in range(NHP):
                            nc.vector.tensor_copy(O_T[hp], K_T[hp][:, 0:SLN])
                    else:
                        with tc.tile_pool(name=f"psa{f}", bufs=1, space="PSUM") as psa:
                            # 3 separate 2-bank score tiles: Tile deps are
                            # whole-tile, so slices of one big tile would
                            # serialize every S-write behind the latest exp.
                            St = [psa.tile([128, 2, CH], F32, name=f"S{f}_{t}",
                                           tag=f"S{t}", padded_shape=[128, 2, 512])
                                  for t in range(3)]
                            Op = psa.tile([128, 2, CH], F32, name=f"Opp{f}", tag="Op",
                                          padded_shape=[128, 2, 512])
                            pending_fin = []

                            def head_attn(g):
                                hp, hh = divmod(g, 2)
                                hs = slice(hh * 64, hh * 64 + 64)
                                aT = at_p.tile([128, KK, SLN], BF16,
                                               name=f"aT{f}_{g}", tag="aT")
                                aTf = aT.rearrange("p kk n -> p (kk n)")

                                def S_mm(l):
                                    kk, cq = divmod(l, 2)
                                    nc.tensor.matmul(
                                        St[(l // 2) % 3][:, l % 2, :],
                                        K_T[hp][hs, kk * 128:(kk + 1) * 128],
                                        Q_T[hp][hs, cq * CH:(cq + 1) * CH],
                                        start=True, stop=True)

                                def E_grp(j):  # exp of S pair l=2j..2j+1
                                    nc.scalar.activation(
                                        aTf[:, 2 * j * CH:(2 * j + 2) * CH],
                                        St[j % 3][:, :, :],
                                        EXP, scale=0.125)

                                def AV(kk):
                                    for cq in range(2):
                                        nc.tensor.matmul(
                                            Op[0:65, cq, :],
                                            V[kk][:, g * 65:(g + 1) * 65],
                                            aT[:, kk, cq * CH:(cq + 1) * CH],
                                            start=(kk == 0), stop=(kk == KK - 1))

                                # software pipeline: 3 independent exp chains
                                # (one per score tile); AV(kk) right after
                                # E_{kk+2} so PE always has independent work.
                                for l in range(18):
                                    S_mm(l)
                                    if l % 2 == 1:
                                        E_grp(l // 2)
                                        if l >= 7:
                                            AV(l // 2 - 3)
                                    # drain the finalize of the previous head
                                    # once this head's pipeline is primed
                                    if l == 5 and pending_fin:
                                        pending_fin.pop()()
                                for kk in range(6, KK):
                                    AV(kk)

                                def finalize():
                                    # 1/denom on DVE (fast approx), broadcast
                                    # across the 64 head dims on GpSimd; no
                                    # ACT/PSUM involvement.
                                    rrow = st_p.tile([1, 2, CH], F32,
                                                     name=f"rr{f}_{g}", tag="rr")
                                    recipB = st_p.tile([64, 2, CH], F32,
                                                       name=f"rb{f}_{g}", tag="rb")
                                    nc.vector.reciprocal(
                                        out=rrow, in_=Op[64:65, :, :])
                                    nc.gpsimd.partition_broadcast(recipB, rrow)
                                    nc.vector.tensor_mul(
                                        O_T[hp][hs, :].rearrange("p (c n) -> p c n", n=CH),
                                        Op[0:64, :, :], recipB)

                                pending_fin.append(finalize)

                            for g in range(2 * NHP):
                                head_attn(g)
                            pending_fin.pop()()

                    # ================= output projection =================
                    with tc.tile_pool(name=f"pso{f}", bufs=2, space="PSUM") as pso:
                        for t0, tl in [(0, 128), (128, 128), (256, 128), (384, 128), (512, 64)]:
                            for co in range(2):
                                op = pso.tile([128, 512], F32,
                                              name=f"op{f}_{t0}_{co}", tag="oproj")
                                for hp in range(NHP):
                                    nc.tensor.matmul(
                                        op[0:tl, :], O_T[hp][:, t0:t0 + tl],
                                        wo[hp // 2][:, hp % 2, co * 512:(co + 1) * 512],
                                        start=(hp == 0), stop=False)
                                # fold the bias in as ones^T @ bo
                                nc.tensor.matmul(
                                    op[0:tl, :], ones128[:, 0:tl],
                                    bo_sb[:, co * 512:(co + 1) * 512],
                                    start=False, stop=True)
                                outst = outst_p.tile([128, 512], F32,
                                                     name=f"os{f}_{t0}_{co}", tag="os")
                                nc.vector.tensor_copy(outst[0:tl, :], op[0:tl, :])
                                nc.sync.dma_start(
                                    out=out_d[f * SLN + t0: f * SLN + t0 + tl,
                                              co * 512:(co + 1) * 512],
                                    in_=outst[0:tl, :])

            if iters > 1:
                with tc.For_i(0, iters, 1):
                    body()
            else:
                body()

    nc.compile()
    return nc


_nc_cache = {}


def _get_nc(iters=1, skip=()):
    key = (iters, tuple(skip))
    if key not in _nc_cache:
        _nc_cache[key] = _build_nc(iters, skip)
    return _nc_cache[key]


def _host_prep(img, freqs_cos, freqs_sin, Wq, Wk, Wv, Wo, bo):
    import ml_dtypes

    BF = ml_dtypes.bfloat16
    img = np.asarray(img, dtype=np.float32)
    freqs_cos = np.asarray(freqs_cos, dtype=np.float32)
    freqs_sin = np.asarray(freqs_sin, dtype=np.float32)

    # pair-swapped sin: sinsw[2i] = sin[2i+1], sinsw[2i+1] = sin[2i]
    sw = np.arange(HD).reshape(-1, 2)[:, ::-1].reshape(-1)
    sinsw = freqs_sin[:, sw]

    cos_f = freqs_cos.reshape(T, N, HD)
    sin_f = sinsw.reshape(T, N, HD)

    # rot(x)[2i] = -x[2i+1], rot(x)[2i+1] = x[2i]; as rot = P @ x (per 64-dim
    # head half, tiled to 128); the matmul takes P^T as the stationary side.
    P = np.zeros((128, 128), np.float32)
    for i in range(64):
        P[2 * i, 2 * i + 1] = -1.0
        P[2 * i + 1, 2 * i] = 1.0
    pT = np.ascontiguousarray(P.T)

    wqT = np.ascontiguousarray(np.asarray(Wq, np.float32).T).astype(BF)
    wkT = np.ascontiguousarray(np.asarray(Wk, np.float32).T).astype(BF)
    wvT = np.ascontiguousarray(np.asarray(Wv, np.float32).T).astype(BF)
    woT = np.ascontiguousarray(np.asarray(Wo, np.float32).T).astype(BF)
    bo2 = np.asarray(bo, dtype=np.float32).reshape(1, C)

    in_maps = []
    for core in range(NCORES):
        b, fp = divmod(core, 4)
        own0, own1 = 2 * fp, 2 * fp + 1
        lo = own0 - 1 if fp > 0 else 1
        hi = own1 + 1 if fp < 3 else 6
        slots = [lo, own1, own0, hi]
        xT = np.ascontiguousarray(img[b, slots].reshape(4 * N, C).T).astype(BF)
        cosT = np.ascontiguousarray(cos_f[slots].reshape(4 * N, HD).T)
        sinT = np.ascontiguousarray(sin_f[slots].reshape(4 * N, HD).T)
        in_maps.append({
            "xT": xT,
            "cosT": np.concatenate([cosT, cosT], axis=0).astype(BF),
            "sinswT": np.concatenate([sinT, sinT], axis=0).astype(BF),
            "wqT": wqT, "wkT": wkT, "wvT": wvT, "woT": woT,
            "pT": pT, "bo": bo2,
        })
    return in_maps


def kernel(img, freqs_cos, freqs_sin, Wq, Wk, Wv, Wo, bo, _iters=1, _skip=()):
    in_maps = _host_prep(img, freqs_cos, freqs_sin, Wq, Wk, Wv, Wo, bo)
    nc = _get_nc(_iters, _skip)
    res = run_bass_kernel_spmd(nc, in_maps, core_ids=list(range(NCORES)))
    out = np.zeros((B, T, N, C), np.float32)
    for core in range(NCORES):
        b, fp = divmod(core, 4)
        r = res.results[core]["out"].reshape(2, N, C)
        out[b, 2 * fp] = r[0]
        out[b, 2 * fp + 1] = r[1]
    return out
